# revision 37
# baseline (speedup 1.0000x reference)
"""Causal self-attention (B=4, T=2048, C=1024, H=16, D=64) on 8 TRN2 NeuronCores.

Sharding: 8 cores = 4 batches x 2 head-groups (8 heads each). Each core:
  - QKV projection for its (batch, head-group) column slice of w_attn,
    producing qT/kT in head-PAIR layout (pair p: head 2p on partitions 0-63,
    head 2p+1 on partitions 64-127, both in [d, t] orientation) and v in
    [t, d] with an appended ones-column for softmax denominators.
  - Causal attention in scoresT layout with ROW-TILED head-pair packing:
    the two heads of a pair run as concurrent K=64 matmuls on disjoint
    row-groups of the PE array (tile_position (0,0) / (64,0)), which doubles
    QK throughput. Scores for both heads land side by side in one PSUM tile
    so a single ACTIVATE exponentiates the pair.
  - Row-sharded output projection -> per-core partial [T, C].
Host sums the two partials per batch and adds b_proj.

All matmul operands are bf16 with fp32 PSUM accumulation. QKV biases are
folded into the PSUM-evacuation DVE ops (tensor_scalar / tensor_tensor), not
matmuls. Input DMAs are spread over four engines' HWDGE queues; a dummy
matmul burst warms the PE HAM clock and a dummy exp preloads the ACT table
during the initial DMA wait. A post-compile peephole drops LDWEIGHTS
instructions that reload the stationary operand already resident in the PE.
"""

import sys
import types

import numpy as np

B, T, C, H, D = 4, 2048, 1024, 16, 64
HG = 8            # heads per core
CG = HG * D       # 512 channels per group
NCORES = 8
TB = T // 128     # 16 t-blocks


def _register_ntff_hook():
    """Register the axon NTFF profile hook if the image's antenv lacks it."""
    try:
        import antenv
        if getattr(antenv, "axon_hooks", None) is not None:
            return
        from trn_agent_boot.trn_boot import _ntff_profile_via_ctypes
        hook = _ntff_profile_via_ctypes("/opt/axon/libaxon_pjrt.so")
        mod = types.ModuleType("antenv.axon_hooks")
        mod._hook = hook
        mod.get_axon_ntff_profile_hook = lambda: mod._hook
        mod.set_axon_ntff_profile_hook = lambda h: setattr(mod, "_hook", h)
        sys.modules["antenv.axon_hooks"] = mod
        antenv.axon_hooks = mod
    except Exception:
        pass


_NC_CACHE = {}


def _dedupe_ldweights(nc):
    """Drop InstLdweights whose weights AP is identical to the previous
    (wait-free) Ldweights on the PE queue with no intervening PE-side sync.

    Cross-engine ordering is enforced by PE-side waits: any producer that
    rewrites a weights region forces a wait on the consuming Ldweights (or
    its matmul), so a wait-free Ldweights with an identical AP is a pure
    reload of the already-resident stationary operand. Other engines'
    instructions interleaved in the block don't touch the PE weight array.
    """
    import concourse.mybir as mybir

    PE = mybir.EngineType.PE
    removed = 0
    for blk in nc.m.functions[0].blocks:
        insts = blk.instructions
        last_key = None
        to_remove = []
        for i, inst in enumerate(insts):
            nm = type(inst).__name__
            if nm == "InstLdweights":
                si = inst.sync_info
                has_sync = si is not None and (
                    len(si.on_wait) > 0 or len(si.on_update) > 0)
                key = repr(inst.ins[0])
                if not has_sync and key == last_key:
                    to_remove.append(i)
                else:
                    last_key = key
            elif nm == "InstMatmult":
                pass
            elif getattr(inst, "engine", None) == PE:
                last_key = None
        for i in reversed(to_remove):
            del insts[i]
        removed += len(to_remove)
    return removed


def _build():
    import concourse.bacc as bacc
    import concourse.mybir as mybir
    import concourse.tile as tile
    from concourse.masks import make_upper_triangular
    from contextlib import ExitStack

    F32 = mybir.dt.float32
    F32R = mybir.dt.float32r
    BF16 = mybir.dt.bfloat16
    MUL = mybir.AluOpType.mult
    ADD = mybir.AluOpType.add
    EXP = mybir.ActivationFunctionType.Exp

    nc = bacc.Bacc(None, target_bir_lowering=False, debug=False)
    dbg = _NC_CACHE.get("debug_yT", False)
    if dbg:
        ydbg_d = nc.dram_tensor("ydbg", [128, 4 * T], BF16, kind="ExternalOutput")
    xT_d = nc.dram_tensor("xT", [C, T], BF16, kind="ExternalInput")
    wqk_d = nc.dram_tensor("wqk", [C, 2 * CG], BF16, kind="ExternalInput")
    wv_d = nc.dram_tensor("wv", [C, CG], BF16, kind="ExternalInput")
    wp_d = nc.dram_tensor("wp", [CG, C], BF16, kind="ExternalInput")
    bqk_d = nc.dram_tensor("bqk", [128, 8], F32, kind="ExternalInput")
    bv_d = nc.dram_tensor("bv", [128, CG], BF16, kind="ExternalInput")
    out_d = nc.dram_tensor("out", [T, C], F32, kind="ExternalOutput")

    CT = C // 128  # 8 c-tiles of the contraction dim

    with tile.TileContext(nc) as tc, ExitStack() as ctx:
        pers = ctx.enter_context(tc.tile_pool(name="pers", bufs=1))

        # Head-pair q/k tiles: pair p holds head 2p on partitions 0-63 and
        # head 2p+1 on partitions 64-127, both as [d, t].
        qp = [pers.tile([128, T], BF16, name=f"qp{p}") for p in range(4)]
        kp = [pers.tile([128, T], BF16, name=f"kp{p}") for p in range(4)]
        # v_aug[p, j, h, 0:64] = v[t=j*128+p, h*64+d]; [..., 64] = 1.0
        v_aug = pers.tile([128, TB, HG, 65], BF16, name="v_aug")
        utri = pers.tile([128, 128], BF16, name="utri")
        ones_col = pers.tile([1, 64], BF16, name="ones_col")
        wz = pers.tile([128, 512], BF16, name="wz")
        bqk_sb = pers.tile([128, 8], F32, name="bqk_sb")
        bv_sb = pers.tile([128, CG], BF16, name="bv_sb")

        stage = pers.tile([128, 128], F32, name="stage")
        make_upper_triangular(nc, utri[:, :], val=1.0, diag=True)
        nc.vector.memset(wz[:], 0.0)
        nc.vector.memset(stage[:], 1.0)
        nc.vector.tensor_copy(ones_col[:], stage[0:1, 0:64])
        nc.vector.tensor_copy(
            v_aug[:, :, :, 64:65],
            stage[:, 0:128].rearrange("p (j h) -> p j h", j=TB))
        nc.sync.dma_start(bqk_sb[:], bqk_d.ap()[:])
        nc.sync.dma_start(bv_sb[:], bv_d.ap()[:])

        wp_pool = ctx.enter_context(tc.tile_pool(name="wp_pool", bufs=1))
        wp_sb = [wp_pool.tile([128, C], BF16, name=f"wp{i}") for i in range(4)]
        yT_pool = ctx.enter_context(tc.tile_pool(name="yT_pool", bufs=1))
        yT = [yT_pool.tile([128, T], BF16, name=f"yT{i}") for i in range(4)]

        att_pool = ctx.enter_context(tc.tile_pool(name="att_pool", bufs=6))
        nrm_pool = ctx.enter_context(tc.tile_pool(name="nrm_pool", bufs=4))
        out_pool = ctx.enter_context(tc.tile_pool(name="out_pool", bufs=2))
        # One big PSUM ring (3 x [128,1024] = 6 banks) shared by projection
        # accumulators, attention score tiles, normalize broadcasts and the
        # output-projection accumulators; plus 2 banks of ps_y (one 512-wide
        # q-chunk of both heads of the current pair).
        big_pool = ctx.enter_context(
            tc.tile_pool(name="big_pool", bufs=3, space="PSUM"))
        psy_pool = ctx.enter_context(
            tc.tile_pool(name="psy_pool", bufs=1, space="PSUM"))

        # Phase-1 working pools (released once attention c2=0 fillers end).
        wqk_pool = tc.alloc_tile_pool(name="wqk_pool", bufs=1)
        wv_pool = tc.alloc_tile_pool(name="wv_pool", bufs=1)
        xq_pool = tc.alloc_tile_pool(name="xq_pool", bufs=2)
        wqk_sb = [wqk_pool.tile([128, 2 * CG], BF16, name=f"wqk{c}")
                  for c in range(CT)]
        wv_sb = [wv_pool.tile([128, CG], BF16, name=f"wv{c}") for c in range(CT)]

        dma_eng = [nc.sync, nc.gpsimd, nc.scalar]

        # PE warm-up burst + ACT exp-table preload during the input DMA wait.
        # Warmups write the psy banks (unused until attention) so the big
        # PSUM ring stays free for real phase-1 work.
        warm_att = att_pool.tile([128, 1024], BF16, tag="att")
        for i in range(16):
            pw = psy_pool.tile([65, 512], F32, name=f"psy{i % 2}",
                               tag=f"psy{i % 2}")
            nc.tensor.matmul(pw[:, :], wz[:, 0:65], wz[:],
                             start=True, stop=True)
            if i == 0:
                nc.scalar.activation(warm_att[0:1, 0:16], pw[0:1, 0:16],
                                     EXP, scale=0.125)

        for c in range(CT):
            dma_eng[c % 3].dma_start(
                wv_sb[c][:], wv_d.ap()[c * 128:(c + 1) * 128, :])

        xq_by_q = {}

        def p1_dma(q):
            xq = []
            for c in range(CT):
                xt = xq_pool.tile([128, 512], BF16, name=f"xq{c}", tag=f"xq{c}")
                dma_eng[c % 3].dma_start(
                    xt[:], xT_d.ap()[c * 128:(c + 1) * 128, q * 512:(q + 1) * 512])
                xq.append(xt)
            xq_by_q[q] = xq

        def p1_v_unit(q, tb):
            """V projection for t-block tb of quarter q (x stationary)."""
            xq = xq_by_q[q]
            pv = big_pool.tile([128, 1024], F32, name="pv", tag="big")
            for c in range(CT):
                nc.tensor.matmul(
                    pv[:, 0:CG], xq[c][:, tb * 128:(tb + 1) * 128], wv_sb[c][:],
                    start=(c == 0), stop=(c == CT - 1))
            j = q * 4 + tb
            nc.vector.tensor_tensor(
                out=v_aug[:, j, :, 0:64],
                in0=pv[:, 0:CG].rearrange("p (h d) -> p h d", h=HG),
                in1=bv_sb[:].rearrange("p (h d) -> p h d", h=HG), op=ADD)

        def p1_qk_unit(q, m):
            """Q/K projection M-block m (heads 2(m%4), 2(m%4)+1) of quarter q."""
            xq = xq_by_q[q]
            pqk = big_pool.tile([128, 1024], F32, name="pqk", tag="big")
            for c in range(CT):
                nc.tensor.matmul(
                    pqk[:, 0:512], wqk_sb[c][:, m * 128:(m + 1) * 128], xq[c][:],
                    start=(c == 0), stop=(c == CT - 1))
            dst = qp if m < 4 else kp
            nc.vector.tensor_scalar(
                out=dst[m % 4][:, q * 512:(q + 1) * 512], in0=pqk[:, 0:512],
                scalar1=bqk_sb[:, m:m + 1], scalar2=None, op0=ADD)

        def p1_units(q):
            for tb in range(4):
                yield lambda tb=tb: p1_v_unit(q, tb)
            for m in range(8):
                yield lambda m=m: p1_qk_unit(q, m)

        def attn_unit(hp, c2, qc, j, jmax):
            """Packed QK -> exp -> mask -> AV for head pair hp, 512-wide
            q-chunk (c2, qc), k-block j."""
            jb = slice(j * 128, (j + 1) * 128)
            qbase = c2 * 1024 + qc * 512
            d = max(0, j * 128 - qbase)
            ps = big_pool.tile([128, 1024], F32, name="ps", tag="big")
            # QK: boosted priority keeps the row-tiled A/B pair adjacent in
            # the scheduled PE stream (an interleaved full-row matmul would
            # serialize the pair and lose the 2x row-group concurrency).
            with tc.high_priority(offset=1500):
                for half, base in ((0, 0), (1, 512)):
                    rows = slice(64 * half, 64 * half + 64)
                    nc.tensor.matmul(
                        ps[:, base + d:base + 512], kp[hp][rows, jb],
                        qp[hp][rows, qbase + d:qbase + 512],
                        start=True, stop=True)
            att = att_pool.tile([128, 1024], BF16, tag="att")
            nc.scalar.activation(att[:, d:1024], ps[:, d:1024],
                                 EXP, scale=0.125)
            if qbase <= j * 128 < qbase + 512:
                with tc.high_priority(offset=700):
                    nc.vector.tensor_tensor(
                        out=att[:, d:d + 128], in0=att[:, d:d + 128],
                        in1=utri[:, :], op=MUL)
                    nc.vector.tensor_tensor(
                        out=att[:, 512 + d:512 + d + 128],
                        in0=att[:, 512 + d:512 + d + 128],
                        in1=utri[:, :], op=MUL)
            with tc.high_priority(offset=700):
                for h01 in (0, 1):
                    nc.tensor.matmul(
                        psy_by[h01][:, d:512], v_aug[:, j, 2 * hp + h01, :],
                        att[:, 512 * h01 + d:512 * h01 + 512],
                        start=(j == 0), stop=(j == jmax))

        def normalize(hp, c2, h01, qc):
            """yT[d, q] = psy[d, q] / psy[64, q]; reciprocal on one lane,
            then partition-broadcast on the (otherwise idle) GpSimd engine."""
            psy = psy_by[h01]
            qbase = c2 * 1024 + qc * 512
            with tc.high_priority(offset=700):
                sums = nrm_pool.tile([1, 512], F32, tag="sums")
                nc.vector.tensor_copy(sums[:], psy[64:65, :])
                inv1 = nrm_pool.tile([1, 512], F32, tag="inv1")
                nc.vector.reciprocal_approx_fast(inv1[:], sums[:])
                inv = nrm_pool.tile([64, 512], F32, tag="inv")
                nc.gpsimd.partition_broadcast(inv[:], inv1[:])
                if h01 == 0:
                    nc.vector.tensor_tensor(
                        out=yT[hp][0:64, qbase:qbase + 512], in0=psy[0:64, :],
                        in1=inv[:], op=MUL)
                else:
                    ystg = nrm_pool.tile([64, 512], BF16, tag="ystg")
                    nc.vector.tensor_tensor(
                        out=ystg[:], in0=psy[0:64, :], in1=inv[:], op=MUL)
                    nc.sync.dma_start(
                        yT[hp][64:128, qbase:qbase + 512], ystg[:])

        def proj_unit(tb):
            pp = big_pool.tile([128, 1024], F32, name="pp", tag="big")
            # ct outer, ch inner: adjacent matmuls share the yT stationary.
            for ct in range(4):
                for ch in range(2):
                    nc.tensor.matmul(
                        pp[:, ch * 512:(ch + 1) * 512],
                        yT[ct][:, tb * 128:(tb + 1) * 128],
                        wp_sb[ct][:, ch * 512:(ch + 1) * 512],
                        start=(ct == 0), stop=(ct == 3))
            o_sb = out_pool.tile([128, C], F32, tag="o_sb")
            nc.vector.tensor_copy(o_sb[:], pp[:])
            nc.sync.dma_start(out_d.ap()[tb * 128:(tb + 1) * 128, :], o_sb[:])

        # ---------------- Orchestration ----------------
        psy_by = {}
        p1_dma(0)
        for c in range(CT):
            dma_eng[(c + 2) % 3].dma_start(
                wqk_sb[c][:], wqk_d.ap()[c * 128:(c + 1) * 128, :])
        p1_dma(1)
        # V units first (they only need wv + x, which arrive before wqk).
        for q in (0, 1):
            for tb in range(4):
                p1_v_unit(q, tb)
        for q in (0, 1):
            for m in range(8):
                p1_qk_unit(q, m)
        for i in range(4):
            dma_eng[i % 3].dma_start(wp_sb[i][:], wp_d.ap()[i * 128:(i + 1) * 128, :])

        def attn_chunk(hp, c2, qc, fillers, every):
            """One 512-wide q-chunk of attention for head pair hp, popping a
            filler unit every `every` k-blocks."""
            for h01 in (0, 1):
                psy_by[h01] = psy_pool.tile(
                    [65, 512], F32, name=f"psy{h01}", tag=f"psy{h01}")
            jmax = 8 * c2 + 4 * qc + 3
            for j in range(jmax + 1):
                attn_unit(hp, c2, qc, j, jmax)
                if j % every == every - 1 and fillers:
                    fillers.pop(0)()
            normalize(hp, c2, 0, qc)
            normalize(hp, c2, 1, qc)

        # Attention runs one 512-wide q-chunk across all four head pairs
        # before moving on, so each finished q-range unlocks its output-
        # projection t-blocks for interleaving with the next chunk. Phase-1
        # units MUST all be emitted before attention c2=1 (Tile tracks
        # dependencies in program order, so the q/k/v writes for t >= 1024
        # have to precede the units that read them).
        p1_dma(2)
        fillers = list(p1_units(2))
        emitted_dma3 = False
        for hp in range(4):
            attn_chunk(hp, 0, 0, fillers, 2)
            if not emitted_dma3:
                p1_dma(3)
                fillers += list(p1_units(3))
                emitted_dma3 = True
        for hp in range(4):
            attn_chunk(hp, 0, 1, fillers, 2)
        for u in fillers:
            u()
        xq_pool.release()
        wv_pool.release()
        wqk_pool.release()

        fillers = [lambda tb=tb: proj_unit(tb) for tb in range(8)]
        for hp in range(4):
            attn_chunk(hp, 1, 0, fillers, 6)
        fillers += [lambda tb=tb: proj_unit(tb) for tb in range(8, 12)]
        for hp in range(4):
            attn_chunk(hp, 1, 1, fillers, 6)
        for u in fillers:
            u()
        for tb in range(12, 16):
            proj_unit(tb)
        if dbg:
            for i in range(4):
                nc.sync.dma_start(ydbg_d.ap()[:, i * T:(i + 1) * T], yT[i][:])

    nc.compile()
    _dedupe_ldweights(nc)
    return nc


def _get_nc():
    if "nc" not in _NC_CACHE:
        _register_ntff_hook()
        _NC_CACHE["nc"] = _build()
    return _NC_CACHE["nc"]


def kernel(x, w_attn, b_attn, w_proj, b_proj, _run_kwargs=None):
    import ml_dtypes
    from concourse.bass_utils import run_bass_kernel_spmd

    bf16 = ml_dtypes.bfloat16
    x = np.asarray(x, dtype=np.float32)
    w_attn = np.asarray(w_attn, dtype=np.float32)
    b_attn = np.asarray(b_attn, dtype=np.float32)
    w_proj = np.asarray(w_proj, dtype=np.float32)
    b_proj = np.asarray(b_proj, dtype=np.float32)

    nc = _get_nc()
    in_maps = []
    for core in range(NCORES):
        b, g = divmod(core, 2)
        cols = slice(g * CG, (g + 1) * CG)
        bqk = np.concatenate(
            [b_attn[cols], b_attn[C + g * CG: C + (g + 1) * CG]])
        in_maps.append({
            "xT": np.ascontiguousarray(x[b].T).astype(bf16),
            "wqk": np.concatenate(
                [w_attn[:, cols], w_attn[:, C + g * CG: C + (g + 1) * CG]],
                axis=1).astype(bf16),
            "wv": np.ascontiguousarray(
                w_attn[:, 2 * C + g * CG: 2 * C + (g + 1) * CG]).astype(bf16),
            "wp": np.ascontiguousarray(w_proj[g * CG:(g + 1) * CG, :]).astype(bf16),
            "bqk": np.ascontiguousarray(
                bqk.reshape(8, 128).T).astype(np.float32),
            "bv": np.broadcast_to(
                b_attn[2 * C + g * CG: 2 * C + (g + 1) * CG],
                (128, CG)).astype(bf16),
        })

    res = run_bass_kernel_spmd(nc, in_maps, core_ids=list(range(NCORES)),
                               **(_run_kwargs or {}))
    out = np.empty((B, T, C), dtype=np.float32)
    for b in range(B):
        out[b] = res.results[2 * b]["out"] + res.results[2 * b + 1]["out"] + b_proj
    if _run_kwargs:
        kernel.last_results = res
    return out
